# revision 22
# baseline (speedup 1.0000x reference)
"""Deconv(stride=1) + ReLU -> offset-conv -> DeformConv2d + ReLU, on 8 trn2 cores.

Sharding: data-parallel over (batch, 64-row bands): core c -> batch c//4,
rows 64*(c%4) .. +64.  Each core receives a zero-padded x slab that covers
its band plus halos (conv + max-offset gather reach), so the compiled
program is identical across cores (pure SPMD, no collectives).

v2 pipeline (f16 data path, engines overlapped):
  1. conv1 (ConvTranspose2d == conv with flipped/transposed weights) + ReLU
     via f16 PE matmuls (tap-pairing on K) -> h f16 in SBUF.  Only the
     potentially-out-of-image top/bottom 7 slab rows are rmask-zeroed.
  2. offset conv in [18, px] orientation (f16 matmuls, f32 psum), PE
     transpose to per-pixel layout -> offs_t f32.
  3. Derived math on DVE (f32): sample positions, robust floor/frac,
     validity-masked bilinear weights written corner-interleaved into w4;
     int16 gather indices folded to the dma_gather 16-partition wrapped
     layout.  Runs concurrently with 4 (different engines).
  4. Per tap k: A_k = wdef_k @ h computed column-major ([78 slab rows] on
     PSUM partitions) -> Act-engine copy converts f32 psum to f16 stage
     tile -> contiguous DMA into the flat A_k DRAM image.
  5. Per tap k, per 2048-px block: two dma_gathers fetch the f16 2-pixel
     corner elements for rows y0/y0+1; DVE blend = 2 wide multiplies with
     corner-paired weights (f32) to f16 + 4 packed f16 adds (2x DVE mode)
     accumulating into per-block acc tiles.  Gathers for tap k overlap
     A-production of tap k+1 and the blend of tap k-1.
  6. Per block: bias + ReLU, store pixel-major f16 output.
Host reassembles [16384, 64] per-core outputs into [2, 64, 256, 256] f32.
"""

import numpy as np

# ---------------- geometry (hardcoded for this problem) ----------------
B, C, H, W = 2, 64, 256, 256
KK = 3
NCORES = 8
RB = 64                  # output rows per core
MG = 7                   # gather halo rows (max |dy| ~ 4.8 -> ceil + margin)
HS = RB + 2 * MG         # 78 h-slab rows
XS = HS + 2              # 80 x-slab rows
WP = W + 2               # 258 padded conv width
NPX = RB * W             # 16384 output pixels per core
NCH = NPX // 128         # 128 chunks of 128 px (half-rows)
NSL = HS * W             # 19968 slab pixels
FB = 16.0                # floor bias (keeps pre-floor values positive)
NBLK = 8                 # gather pixel blocks
BPX = NPX // NBLK        # 2048 px per block
APITCH = NSL + 32        # A image pitch in pixel slots (lead zero + tail pad)

_CACHE = {}


def _build():
    import concourse.bass as bass
    import concourse.tile as tile
    from concourse import bacc, mybir

    f32 = mybir.dt.float32
    f16 = mybir.dt.float16
    i16 = mybir.dt.int16
    AF = mybir.ActivationFunctionType
    OP = mybir.AluOpType

    nc = bacc.Bacc("TRN2", num_devices=NCORES)

    # ---- I/O ----
    xs_d = nc.dram_tensor("xs", [64, XS * WP], f16, kind="ExternalInput").ap()
    c1w_d = nc.dram_tensor("c1w", [128, 384], f16, kind="ExternalInput").ap()
    offw_d = nc.dram_tensor("offw", [64, 162], f16, kind="ExternalInput").ap()
    defw_d = nc.dram_tensor("defw", [64, 576], f16, kind="ExternalInput").ap()
    bdc_d = nc.dram_tensor("bdc", [64, 1], f32, kind="ExternalInput").ap()
    basey_d = nc.dram_tensor("basey", [128, NCH * 9], f32, kind="ExternalInput").ap()
    basex_d = nc.dram_tensor("basex", [128, NCH * 9], f32, kind="ExternalInput").ap()
    bdef_d = nc.dram_tensor("bdef", [128, 64], f16, kind="ExternalInput").ap()
    rmask_d = nc.dram_tensor("rmask", [64, 14], f16, kind="ExternalInput").ap()
    out_d = nc.dram_tensor("out", [NPX, 64], f16, kind="ExternalOutput").ap()

    adram = [nc.dram_tensor(f"adram{k}", [APITCH * 64], f32, kind="Internal").ap()
             for k in range(9)]

    with tile.TileContext(nc) as tc:
        _prog(nc, tc, bass, mybir, f32, f16, i16, AF, OP,
              xs_d, c1w_d, offw_d, defw_d, bdc_d, basey_d, basex_d, bdef_d,
              rmask_d, out_d, adram)
    nc.compile()
    return nc


def _prog(nc, tc, bass, mybir, f32, f16, i16, AF, OP,
          xs_d, c1w_d, offw_d, defw_d, bdc_d, basey_d, basex_d, bdef_d,
          rmask_d, out_d, adram):
    from contextlib import ExitStack

    ctx = ExitStack()
    with ctx:
        constsF = ctx.enter_context(tc.tile_pool(name="constsF", bufs=1))
        bdef = constsF.tile([128, 64], f16)
        cw = ctx.enter_context(tc.tile_pool(name="cw", bufs=1))
        c1w = cw.tile([128, 384], f16)
        nc.sync.dma_start(c1w[:], c1w_d)
        offw = cw.tile([64, 162], f16)
        defw = cw.tile([64, 576], f16)
        bdc = cw.tile([64, 1], f32)
        nc.sync.dma_start(bdc[:], bdc_d)

        hpool = ctx.enter_context(tc.tile_pool(name="hpool", bufs=1))
        h = hpool.tile([64, HS * WP], f16)
        drv = ctx.enter_context(tc.tile_pool(name="drv", bufs=1))
        w4 = drv.tile([128, NCH * 36], f32)
        idxF = drv.tile([128, 18432], i16)
        offs_stack = ExitStack()
        offs = offs_stack.enter_context(tc.tile_pool(name="offs", bufs=1))
        offs_t = offs.tile([128, NCH * 18], f32)
        # zero the conv-pad columns (0 and WP-1); interior is fully written
        nc.vector.memset(bass.AP(h.tensor, h.offset, [[HS * WP, 64], [WP, HS]]),
                         0.0)
        nc.vector.memset(
            bass.AP(h.tensor, h.offset + WP - 1, [[HS * WP, 64], [WP, HS]]), 0.0)

        # ---------- phase B: conv1 + relu -> h (x streamed in bands) ----
        with tc.tile_pool(name="xband", bufs=2) as xbp, \
             tc.tile_pool(name="c1psum", bufs=3, space="PSUM") as c1ps:
            for b in range(HS // 6):  # 13 bands of 6 output rows
                xb0 = xbp.tile([128, 8 * WP], f16, name="xb")
                r0 = 6 * b
                nc.sync.dma_start(xb0[0:64, :],
                                  xs_d[:, r0 * WP:(r0 + 8) * WP])
                nc.sync.dma_start(xb0[64:128, 0:7 * WP],
                                  xs_d[:, (r0 + 1) * WP:(r0 + 8) * WP])
                for t in range(3):
                    ps = c1ps.tile([64, 512], f32, name="c1t")
                    y0l = 2 * t
                    for kx in range(3):
                        nc.tensor.matmul(
                            ps[:].rearrange("p (a b) -> p a b", a=2),
                            c1w[:, kx * 64:(kx + 1) * 64],
                            bass.AP(xb0.tensor, xb0.offset + y0l * WP + kx,
                                    [[8 * WP, 128], [WP, 2], [1, W]]),
                            start=(kx == 0), stop=False)
                    for kx in range(3):
                        nc.tensor.matmul(
                            ps[:].rearrange("p (a b) -> p a b", a=2),
                            c1w[0:64, 192 + kx * 64:192 + (kx + 1) * 64],
                            bass.AP(xb0.tensor, xb0.offset + (y0l + 2) * WP + kx,
                                    [[8 * WP, 64], [WP, 2], [1, W]]),
                            start=False, stop=(kx == 2))
                    hout = bass.AP(h.tensor, h.offset + (r0 + y0l) * WP + 1,
                                   [[HS * WP, 64], [WP, 2], [1, W]])
                    nc.scalar.activation(
                        hout, ps[:].rearrange("p (a b) -> p a b", a=2),
                        AF.Relu, bias=bdc[:, 0:1], scale=1.0)

        # deferred loads: not needed until after conv1
        nc.sync.dma_start(offw[:], offw_d)
        nc.sync.dma_start(defw[:], defw_d)
        nc.sync.dma_start(bdef[:], bdef_d)
        with tc.tile_pool(name="zpool", bufs=1) as zp:
            ztile = zp.tile([128, 1024], f32)
            nc.vector.memset(ztile[:], 0.0)
            for k in range(9):
                nc.sync.dma_start(
                    bass.AP(adram[k].tensor, 0, [[1024, 1], [1, 64]]),
                    bass.AP(ztile.tensor, ztile.offset, [[1024, 1], [1, 64]]))
                nc.sync.dma_start(
                    bass.AP(adram[k].tensor, (NSL + 1) * 64,
                            [[992, 2], [1, 992]]),
                    bass.AP(ztile.tensor, ztile.offset, [[1024, 2], [1, 992]]))

        # zero h rows outside the image: only the top/bottom MG rows can be
        # invalid (conv1 bias makes them nonzero otherwise)
        rmask = cw.tile([64, 14], f16)
        nc.sync.dma_start(rmask[:], rmask_d)
        nc.vector.tensor_tensor(
            bass.AP(h.tensor, h.offset, [[HS * WP, 64], [WP, MG], [1, WP]]),
            bass.AP(h.tensor, h.offset, [[HS * WP, 64], [WP, MG], [1, WP]]),
            bass.AP(rmask.tensor, rmask.offset, [[14, 64], [1, MG], [0, WP]]),
            OP.mult)
        nc.vector.tensor_tensor(
            bass.AP(h.tensor, h.offset + (HS - MG) * WP,
                    [[HS * WP, 64], [WP, MG], [1, WP]]),
            bass.AP(h.tensor, h.offset + (HS - MG) * WP,
                    [[HS * WP, 64], [WP, MG], [1, WP]]),
            bass.AP(rmask.tensor, rmask.offset + MG, [[14, 64], [1, MG], [0, WP]]),
            OP.mult)

        # ---------- phase C: offset conv -> offs_t [128, NCH*18] -------
        # pixel-stationary: lhsT = h window [64ch, 128px], rhs = offw taps;
        # psum accumulates the 9 taps; output is already per-pixel layout
        with tc.tile_pool(name="c2psum", bufs=4, space="PSUM") as c2p:
            for u in range(NCH):  # 128 units of 128 px (half rows)
                y0, xb = u // 2, (u % 2) * 128
                po = c2p.tile([128, 18], f32, name="c2o")
                for k in range(9):
                    ky, kx = k // 3, k % 3
                    lhsT = bass.AP(h.tensor,
                                   h.offset + (MG + y0 + ky - 1) * WP + kx + xb,
                                   [[HS * WP, 64], [1, 128]])
                    nc.tensor.matmul(
                        po[:], lhsT, offw[:, k * 18:(k + 1) * 18],
                        start=(k == 0), stop=(k == 8))
                nc.vector.tensor_copy(offs_t[:, u * 18:(u + 1) * 18], po[:])

        from concourse import library_config
        nc.gpsimd.load_library(library_config.mlp)

        # ---------- phase E: derived math (overlaps phase D on DVE) ----
        idx_stack = ExitStack()
        idxa = idx_stack.enter_context(tc.tile_pool(name="idxa", bufs=1))
        idx_all = idxa.tile([128, 2304], i16)
        with tc.tile_pool(name="escratch", bufs=1) as esc:
            basey = esc.tile([128, NCH * 9], f32)
            nc.sync.dma_start(basey[:], basey_d)
            basex = esc.tile([128, NCH * 9], f32)
            nc.sync.dma_start(basex[:], basex_d)
            ty = esc.tile([128, NCH * 9], f32)
            tx = esc.tile([128, NCH * 9], f32)
            iyf = esc.tile([128, NCH * 9], f32)
            ixf = esc.tile([128, NCH * 9], f32)
            fy = esc.tile([128, NCH * 9], f32)
            fx = esc.tile([128, NCH * 9], f32)
            tmp = esc.tile([128, NCH * 9], f32)

            def oview(off):  # offsets strided view: [128, chunk, 9] step 2
                return bass.AP(offs_t.tensor, offs_t.offset + off,
                               [[NCH * 18, 128], [18, NCH], [2, 9]])

            def w4v(cr):  # w4 corner-strided write view [128, chunk, 9]
                return bass.AP(w4.tensor, w4.offset + cr,
                               [[NCH * 36, 128], [36, NCH], [4, 9]])

            t3 = lambda t: t[:].rearrange("p (a b) -> p a b", a=NCH)
            nc.vector.tensor_tensor(t3(ty), oview(0), t3(basey), OP.add)
            nc.vector.tensor_tensor(t3(tx), oview(1), t3(basex), OP.add)

            M23 = 8388608.0  # 2^23: (t+M)-M rounds t to the nearest integer

            def floorify(t, intf, frac):
                nc.vector.tensor_scalar(intf[:], t[:], M23, -M23, OP.add, OP.add)
                nc.vector.tensor_tensor(frac[:], t[:], intf[:], OP.subtract)
                nc.vector.tensor_scalar(tmp[:], frac[:], 0.0, None, OP.is_lt)
                nc.vector.tensor_tensor(intf[:], intf[:], tmp[:], OP.subtract)
                nc.vector.tensor_tensor(frac[:], t[:], intf[:], OP.subtract)

            floorify(ty, iyf, fy)
            floorify(tx, ixf, fx)

            # idx path first -- it gates the gathers (weights gate only blends)
            # idx = (iyf-FB)*W + (ixf-16) + 1, clamped to [0, NSL]
            nc.vector.tensor_scalar(tmp[:], iyf[:], float(W),
                                    -(FB * W + 16.0 - 1.0), OP.mult, OP.add)
            nc.vector.tensor_tensor(ty[:], tmp[:], ixf[:], OP.add)
            nc.vector.tensor_scalar(ty[:], ty[:], 0.0, float(NSL), OP.max, OP.min)
            ia4 = idx_all[:].rearrange("p (a b c) -> p a b c", a=NCH, b=9)
            iv = lambda cr: ia4[:, :, :, cr]
            nc.vector.tensor_copy(iv(0), t3(ty))
            nc.vector.tensor_scalar(ty[:], ty[:], float(W), float(NSL),
                                    OP.add, OP.min)
            nc.vector.tensor_copy(iv(1), t3(ty))

            # fold indices to the 16-partition wrapped layout:
            # 1) partition-fold 128->16 rows via 8 contiguous SBUF DMAs
            # 2) column-interleave within partitions via DVE strided copies
            # 3) replicate rows 0-15 to all 8 16-row groups via 7 DMAs
            with tc.tile_pool(name="foldp", bufs=1) as fp:
                tmp16 = fp.tile([16, 8 * 2304], i16)
                for jm in range(8):
                    nc.sync.dma_start(tmp16[0:16, jm * 2304:(jm + 1) * 2304],
                                      idx_all[16 * jm:16 * (jm + 1), :])
                iF5 = idxF[:].rearrange("p (a b c d) -> p a b c d",
                                        a=8, b=18, c=16)
                tm5 = tmp16[:].rearrange("p (jm blk jc kcr) -> p blk kcr jc jm",
                                         jm=8, blk=8, jc=16, kcr=18)
                for blk in range(8):
                    nc.vector.tensor_copy(iF5[0:16, blk], tm5[:, blk])
                for r in range(1, 8):
                    nc.sync.dma_start(idxF[16 * r:16 * (r + 1), :],
                                      idxF[0:16, :])

                # bilinear corner weights (off the idx critical path)
                # x-corner validity (A has no x pads): ix0 = ixf-16 must be
                # in [0,255] for the x0 corner, [-1,254] for x1
                vx = tx  # tx is dead after floorify
                nc.vector.tensor_scalar(vx[:], ixf[:], 16.0, None, OP.is_ge)
                nc.vector.tensor_scalar(tmp[:], ixf[:], 271.0, None, OP.is_le)
                nc.vector.tensor_tensor(vx[:], vx[:], tmp[:], OP.mult)
                # wx0 = (1-fx)*vx0
                nc.vector.tensor_scalar(ty[:], fx[:], -1.0, 1.0, OP.mult, OP.add)
                nc.vector.tensor_tensor(ty[:], ty[:], vx[:], OP.mult)
                nc.vector.tensor_scalar(vx[:], ixf[:], 15.0, None, OP.is_ge)
                nc.vector.tensor_scalar(tmp[:], ixf[:], 270.0, None, OP.is_le)
                nc.vector.tensor_tensor(vx[:], vx[:], tmp[:], OP.mult)
                # wx1 = fx*vx1
                nc.vector.tensor_tensor(fx[:], fx[:], vx[:], OP.mult)
                # corner weights, interleaved [chunk, 9, 4]
                nc.vector.tensor_scalar(tmp[:], fy[:], -1.0, 1.0, OP.mult,
                                        OP.add)
                nc.vector.tensor_tensor(w4v(0), t3(tmp), t3(ty), OP.mult)
                nc.vector.tensor_tensor(w4v(1), t3(tmp), t3(fx), OP.mult)
                nc.vector.tensor_tensor(w4v(2), t3(fy), t3(ty), OP.mult)
                nc.vector.tensor_tensor(w4v(3), t3(fy), t3(fx), OP.mult)
        idx_stack.close()
        offs_stack.close()

        # ---------- phases D+F interleaved per tap ----------
        accp = ctx.enter_context(tc.tile_pool(name="accp", bufs=1))
        acc_all = accp.tile([128, NBLK * 1024], f16)
        reg_bpx = nc.gpsimd.to_reg(BPX)

        GC = 16  # columns per A-production group
        gpool = ctx.enter_context(tc.tile_pool(name="gpool", bufs=3))
        mpool = ctx.enter_context(tc.tile_pool(name="mpool", bufs=3))
        tpool = ctx.enter_context(tc.tile_pool(name="tpool", bufs=2))
        outp = ctx.enter_context(tc.tile_pool(name="out_p", bufs=2))
        with tc.tile_pool(name="astage", bufs=3) as astage, \
             tc.tile_pool(name="apsum", bufs=3, space="PSUM") as aps:
            for k in range(9):
                # --- produce A_k (PE + Act + DMA) ---
                for g in range(W // GC):
                    pa = aps.tile([78, GC * 64], f32, name="apt")
                    for jj in range(GC):
                        j = g * GC + jj
                        lhsT = bass.AP(h.tensor, h.offset + 1 + j,
                                       [[HS * WP, 64], [WP, HS]])
                        nc.tensor.matmul(pa[:, jj * 64:(jj + 1) * 64],
                                         lhsT, defw[:, k * 64:(k + 1) * 64],
                                         start=True, stop=True)
                    st = astage.tile([78, GC * 64], f32, name="ast")
                    nc.scalar.activation(st[:], pa[:], AF.Copy)
                    nc.sync.dma_start(
                        bass.AP(adram[k].tensor, (1 + g * GC) * 64,
                                [[W * 64, 78], [1, GC * 64]]),
                        st[:])

                # --- gather + blend for tap k (DMA/Pool + DVE) ---
                ain_k = bass.AP(adram[k].tensor, 0, [[64, APITCH - 1], [1, 128]])
                for blk in range(NBLK):
                    g0 = gpool.tile([128, 2048], f32, name="g0")
                    g1 = gpool.tile([128, 2048], f32, name="g1")
                    ibase = ((blk * 9 + k) * 2) * 128
                    nc.gpsimd.dma_gather(
                        g0[:].rearrange("p (a b) -> p a b", a=16), ain_k,
                        idxF[:, ibase:ibase + 128], BPX, reg_bpx, 128,
                        elem_step=64, single_packet=False)
                    nc.gpsimd.dma_gather(
                        g1[:].rearrange("p (a b) -> p a b", a=16), ain_k,
                        idxF[:, ibase + 128:ibase + 256], BPX, reg_bpx, 128,
                        elem_step=64, single_packet=False)

                    gv = lambda t: bass.AP(t.tensor, t.offset,
                                           [[2048, 128], [128, 16], [64, 2],
                                            [1, 64]])
                    wv = lambda cr: bass.AP(
                        w4.tensor, w4.offset + (blk * 16) * 36 + k * 4 + cr,
                        [[NCH * 36, 128], [36, 16], [1, 2], [0, 64]])
                    m0 = mpool.tile([128, 2048], f16, name="m0")
                    m1 = mpool.tile([128, 2048], f16, name="m1")
                    nc.vector.tensor_tensor(gv(m0), gv(g0), wv(0), OP.mult)
                    nc.vector.tensor_tensor(gv(m1), gv(g1), wv(2), OP.mult)
                    half = lambda t, off: bass.AP(
                        t.tensor, t.offset + off,
                        [[2048, 128], [128, 16], [1, 64]])
                    a3 = lambda t: t[:].rearrange("p (a b) -> p a b", a=16)
                    t0 = tpool.tile([128, 1024], f16, name="t0")
                    t1 = tpool.tile([128, 1024], f16, name="t1")
                    nc.vector.tensor_tensor(a3(t0), half(m0, 0), half(m0, 64),
                                            OP.add)
                    nc.vector.tensor_tensor(a3(t1), half(m1, 0), half(m1, 64),
                                            OP.add)
                    av = bass.AP(acc_all.tensor, acc_all.offset + blk * 1024,
                                 [[NBLK * 1024, 128], [64, 16], [1, 64]])
                    if k == 0:
                        nc.vector.tensor_tensor(av, a3(t0), a3(t1), OP.add)
                    else:
                        nc.vector.tensor_tensor(av, av, a3(t0), OP.add)
                        nc.vector.tensor_tensor(av, av, a3(t1), OP.add)
                    if k == 8:
                        ob = outp.tile([128, 1024], f16, name="ob")
                        bv = bass.AP(bdef.tensor, bdef.offset,
                                     [[64, 128], [0, 16], [1, 64]])
                        nc.vector.tensor_tensor(a3(ob), av, bv, OP.add)
                        nc.vector.tensor_scalar(ob[:], ob[:], 0.0, None, OP.max)
                        od = bass.AP(out_d.tensor, blk * BPX * 64,
                                     [[1024, 128], [64, 16], [1, 64]])
                        nc.sync.dma_start(od, ob[:])



def _host_prep(x, w_deconv, b_deconv, w_off, b_off, w_def, b_def):
    f = np.float32
    h16 = np.float16
    wt = np.flip(w_deconv, (2, 3)).transpose(1, 0, 2, 3).astype(f)  # [o,c,ky,kx]

    c1w = np.zeros((128, 384), f)
    for kx in range(3):
        c1w[0:64, kx * 64:(kx + 1) * 64] = wt[:, :, 0, kx].T
        c1w[64:128, kx * 64:(kx + 1) * 64] = wt[:, :, 1, kx].T
        c1w[0:64, 192 + kx * 64:192 + (kx + 1) * 64] = wt[:, :, 2, kx].T

    offw = np.zeros((64, 162), f)
    for k in range(9):
        ky, kx = k // 3, k % 3
        offw[:, k * 18:(k + 1) * 18] = w_off[:, :, ky, kx].T  # [c, 18]

    defw = np.zeros((64, 576), f)
    for k in range(9):
        ky, kx = k // 3, k % 3
        defw[:, k * 64:(k + 1) * 64] = w_def[:, :, ky, kx].T  # [c, o]

    bdc = b_deconv.reshape(64, 1).astype(f)
    bdef = np.broadcast_to(b_def.reshape(1, 64), (128, 64)).astype(h16).copy()

    basey = np.zeros((128, NCH, 9), f)
    basex = np.zeros((128, NCH, 9), f)
    p = np.arange(128)
    for ch in range(NCH):
        y = ch // 2
        xcol = (ch % 2) * 128 + p
        for k in range(9):
            ky, kx = k // 3, k % 3
            basey[:, ch, k] = (MG + y + ky - 1) + b_off[2 * k] + FB
            basex[:, ch, k] = xcol + (kx - 1) + b_off[2 * k + 1] + 16.0
    basey = basey.reshape(128, NCH * 9)
    basex = basex.reshape(128, NCH * 9)

    xsl, rml = [], []
    for c in range(NCORES):
        b, r0 = c // 4, RB * (c % 4)
        xs = np.zeros((64, XS, WP), h16)
        glo = r0 - MG - 1          # image row of slab row 0
        lo, hi = max(0, glo), min(H, glo + XS)
        if hi > lo:
            xs[:, lo - glo:hi - glo, 1:W + 1] = x[b, :, lo:hi, :]
        xsl.append(np.ascontiguousarray(xs.reshape(64, XS * WP)))
        hrows = np.arange(HS) + (r0 - MG)      # image row of each h slab row
        rm = ((hrows >= 0) & (hrows < H)).astype(h16)
        rme = np.concatenate([rm[0:MG], rm[HS - MG:HS]])  # top 7 + bottom 7
        rml.append(np.broadcast_to(rme[None, :], (64, 14)).copy())

    shared = dict(c1w=c1w.astype(h16), offw=offw.astype(h16),
                  defw=defw.astype(h16), bdc=bdc,
                  basey=basey, basex=basex, bdef=bdef)
    return xsl, rml, shared


def kernel(x, w_deconv, b_deconv, w_off, b_off, w_def, b_def):
    from concourse.bass_utils import run_bass_kernel_spmd

    if "nc" not in _CACHE:
        _CACHE["nc"] = _build()
    nc = _CACHE["nc"]

    xsl, rml, shared = _host_prep(np.asarray(x), np.asarray(w_deconv),
                                  np.asarray(b_deconv), np.asarray(w_off),
                                  np.asarray(b_off), np.asarray(w_def),
                                  np.asarray(b_def))
    in_maps = [dict(xs=xsl[c], rmask=rml[c], **shared) for c in range(NCORES)]
    res = run_bass_kernel_spmd(nc, in_maps, core_ids=list(range(NCORES)))

    out = np.zeros((B, 64, H, W), np.float32)
    for c in range(NCORES):
        b, r0 = c // 4, RB * (c % 4)
        o = res.results[c]["out"].astype(np.float32)
        o = o.reshape(NBLK, 128, 16, 64).transpose(0, 2, 1, 3).reshape(RB, W, 64)
        out[b, :, r0:r0 + RB, :] = o.transpose(2, 0, 1)
    return out
